# revision 1
# baseline (speedup 1.0000x reference)
"""EdgeConv (gnn_message_passing) Trainium2 Bass kernel.

Computation (reference):
    neigh = x[ind]                                   # [n, k, d] gather
    feat  = [neigh - center, center]                 # [n, k, 2d]
    h     = relu(feat @ W1 + b1) @ W2 + b2           # [n, k, H]
    out   = max over k                               # [n, H]

Algebraic restructuring used here:
    feat @ W1 = neigh @ W1[:d] + center @ (W1[d:] - W1[:d])
so the kernel builds slab = [neigh | center] (no subtraction needed) and a
re-packed weight W1' = [[W1[:d]], [W1[d:] - W1[:d]]], turning layer 1 into a
single K=128 matmul.  b2 is added after the max (max(h)+b2 == max(h+b2)).

Per-core dataflow (data-parallel over points, 8 cores):
  - x is cast to bf16 once on host and replicated; neighbors are fetched with
    a GPSIMD indirect DMA gather (128B rows) into an edge-major slab.
  - centers are staged with small DMAs and broadcast across partitions with a
    DVE stream_shuffle into the slab's other half.
  - one HWDGE xbar DMA-transpose per megablock converts the slab to
    feature-major [128, T, 128] for the tensor engine.
  - PE: matmul1 (W1' stationary) -> ACT relu+b1 -> PE matmul2 -> DVE
    tensor_reduce(max) over k=16 -> +b2 -> DMA out.
  - output is produced transposed ([H, points]); the host transposes back.
"""

import os
import sys

for _p in ("/opt/trn_rl_repo",):
    if _p not in sys.path and os.path.isdir(_p):
        sys.path.insert(0, _p)

import numpy as np
import ml_dtypes

BF16 = ml_dtypes.bfloat16

# problem constants (hardcoded per harness contract)
N, D, K, H = 100000, 64, 16, 128
NCORES = 8
NP = 12500            # points per core
MEGA = 512            # points per megablock
SUB = 8               # points per 128-edge subgroup (128 / K)


class Cfg:
    def __init__(self, n=N, np_=NP, mega=MEGA):
        self.n = n                      # rows of x
        self.np = np_                   # points handled by this core (unpadded)
        self.mega = mega                # points per megablock
        self.t = mega // SUB            # 128-edge subgroups per megablock
        self.nmega = -(-np_ // mega)    # ceil
        self.npp = self.nmega * mega    # padded points per core
        assert mega % 32 == 0


def build_program(cfg: Cfg, debug=False, dump=False):
    """Build the per-core Bass program (same program for every core).

    dump=True adds debug DRAM outputs capturing m=0 intermediates.
    """
    import concourse.bacc as bacc
    import concourse.bass as bass
    import concourse.tile as tile
    from concourse import mybir

    f32 = mybir.dt.float32
    bf16 = mybir.dt.bfloat16
    i32 = mybir.dt.int32
    T = cfg.t

    nc = bacc.Bacc("TRN2", target_bir_lowering=False, debug=debug)

    x2 = nc.dram_tensor("x2", (cfg.n, D), bf16, kind="ExternalInput")
    indl = nc.dram_tensor("indl", (128, cfg.nmega * T), i32, kind="ExternalInput")
    xst = nc.dram_tensor("xst", (8, cfg.nmega, T, D), bf16, kind="ExternalInput")
    w1 = nc.dram_tensor("w1", (2 * D, H), bf16, kind="ExternalInput")
    w2 = nc.dram_tensor("w2", (H, H), bf16, kind="ExternalInput")
    b1 = nc.dram_tensor("b1", (H, 1), f32, kind="ExternalInput")
    b2 = nc.dram_tensor("b2", (H, 1), f32, kind="ExternalInput")
    out2 = nc.dram_tensor("out2", (H, cfg.npp), f32, kind="ExternalOutput")
    if dump:
        d_slab = nc.dram_tensor("d_slab", (128, T * 2 * D), bf16,
                                kind="ExternalOutput")
        d_slabT = nc.dram_tensor("d_slabT", (128, T * 2 * D), bf16,
                                 kind="ExternalOutput")
        d_h1 = nc.dram_tensor("d_h1", (H, 512), bf16, kind="ExternalOutput")

    # lanes 0..31 <- lane (l // 16) within each 32-partition quadrant
    shuf_mask = [i // 16 for i in range(32)]

    with tile.TileContext(nc) as tc:
        with (
            tc.tile_pool(name="const", bufs=1) as constp,
            tc.tile_pool(name="off", bufs=3) as offp,
            tc.tile_pool(name="slab", bufs=2) as slabp,
            tc.tile_pool(name="slabT", bufs=2) as slabTp,
            tc.tile_pool(name="h1", bufs=4) as h1p,
            tc.tile_pool(name="mx", bufs=2) as mxp,
            tc.tile_pool(name="outs", bufs=2) as outp,
            tc.tile_pool(name="ps1", bufs=4, space="PSUM") as ps1p,
            tc.tile_pool(name="ps2", bufs=4, space="PSUM") as ps2p,
        ):
            # persistent double-buffered center staging tiles; memset once so
            # the stream_shuffle's full-partition read is fully initialized
            # padded to [.., 2*D] so the shuffle's in/out APs have identical
            # stride structure (both [128, T, D] strided views)
            l2s = []
            for i in range(2):
                t_ = constp.tile([128, T, 2 * D], bf16, tag=f"l2buf{i}")
                nc.vector.memset(t_[:], 0)
                l2s.append(t_)

            w1s = constp.tile([2 * D, H], bf16)
            nc.sync.dma_start(w1s[:], w1[:, :])
            w2s = constp.tile([H, H], bf16)
            nc.sync.dma_start(w2s[:], w2[:, :])
            b1s = constp.tile([H, 1], f32)
            nc.sync.dma_start(b1s[:], b1[:, :])
            b2s = constp.tile([H, 1], f32)
            nc.sync.dma_start(b2s[:], b2[:, :])

            for m in range(cfg.nmega):
                off = offp.tile([128, T], i32)
                nc.sync.dma_start(off[:], indl[:, m * T:(m + 1) * T])

                # stage center rows: partition 32*g + s holds point rows
                # (mega*m + 8*t + 2*g + s) over free slots t
                l2 = l2s[m % 2]
                for s in range(2):
                    for g in range(4):
                        nc.sync.dma_start(
                            l2[32 * g + s:32 * g + s + 1, :, 0:D],
                            xst[4 * s + g:4 * s + g + 1, m, :, :],
                        )

                slab = slabp.tile([128, T, 2 * D], bf16)
                # HW indirect DMA supports one offset per partition per call
                # (multi-offset APs return garbage on HW), so gather each
                # 128-edge subgroup separately.
                for t in range(T):
                    nc.gpsimd.indirect_dma_start(
                        out=slab[:, t, 0:D],
                        out_offset=None,
                        in_=x2[:, :],
                        in_offset=bass.IndirectOffsetOnAxis(
                            ap=off[:, t:t + 1], axis=0),
                    )
                nc.vector.stream_shuffle(
                    slab[:, :, D:2 * D], l2[:, :, 0:D], shuf_mask
                )

                slabT = slabTp.tile([128, T, 2 * D], bf16)
                nc.sync.dma_start_transpose(
                    slabT[:], slab[:].rearrange("p a b -> p (a b)")
                )
                if dump and m == 0:
                    nc.sync.dma_start(
                        d_slab[:, :], slab[:].rearrange("p a b -> p (a b)"))
                    nc.sync.dma_start(
                        d_slabT[:, :], slabT[:].rearrange("p a b -> p (a b)"))

                mx = mxp.tile([H, cfg.mega], f32)
                for g in range(T // 4):
                    p1 = ps1p.tile([H, 512], f32)
                    nc.tensor.matmul(
                        p1[:], lhsT=w1s[:], rhs=slabT[:, 4 * g:4 * g + 4, :],
                        start=True, stop=True,
                    )
                    h1 = h1p.tile([H, 512], bf16)
                    nc.scalar.activation(
                        h1[:], p1[:], mybir.ActivationFunctionType.Relu,
                        bias=b1s[:], scale=1.0,
                    )
                    if dump and m == 0 and g == 0:
                        nc.sync.dma_start(d_h1[:, :], h1[:])
                    p2 = ps2p.tile([H, 512], f32)
                    nc.tensor.matmul(
                        p2[:], lhsT=w2s[:], rhs=h1[:], start=True, stop=True,
                    )
                    nc.vector.tensor_reduce(
                        out=mx[:, 32 * g:32 * g + 32],
                        in_=p2[:].rearrange("p (a b) -> p a b", b=K),
                        axis=mybir.AxisListType.X,
                        op=mybir.AluOpType.max,
                    )

                outt = outp.tile([H, cfg.mega], f32)
                nc.vector.tensor_scalar(
                    out=outt[:], in0=mx[:], scalar1=b2s[:], scalar2=None,
                    op0=mybir.AluOpType.add,
                )
                nc.sync.dma_start(
                    out2[:, m * cfg.mega:(m + 1) * cfg.mega], outt[:]
                )

    nc.compile()
    return nc


def host_prep(cfg: Cfg, x, ind, W1, b1, W2, b2):
    """Shared (core-independent) input prep."""
    xb = np.ascontiguousarray(x.astype(BF16))
    what = np.vstack([W1[:D], W1[D:] - W1[:D]]).astype(BF16)
    w2b = W2.astype(BF16)
    b1c = np.ascontiguousarray(b1.astype(np.float32).reshape(H, 1))
    b2c = np.ascontiguousarray(b2.astype(np.float32).reshape(H, 1))
    return xb, what, w2b, b1c, b2c


def core_inputs(cfg: Cfg, xb, what, w2b, b1c, b2c, ind32, lo, hi):
    """Build one core's input map for its point range [lo, hi)."""
    T = cfg.t
    indc = np.zeros((cfg.npp, K), np.int32)
    indc[:hi - lo] = ind32[lo:hi]
    # indl[l, m*T + t] = indc[m*MEGA + 8t + l//16, l%16]
    i4 = indc.reshape(cfg.nmega, T, SUB, K)          # [m, t, u, j]
    indl = np.ascontiguousarray(
        i4.transpose(2, 3, 0, 1).reshape(SUB * K, cfg.nmega * T)
    )
    xc = np.zeros((cfg.npp, D), BF16)
    xc[:hi - lo] = xb[lo:hi]
    x4 = xc.reshape(cfg.nmega, T, SUB, D)            # [m, t, u, c], u = 2g+s
    # slot order: idx = 4*s + g  <-> u = 2*g + s
    perm = [2 * g + s for s in range(2) for g in range(4)]
    xstage = np.ascontiguousarray(x4.transpose(2, 0, 1, 3)[perm])
    return {
        "x2": xb, "indl": indl, "xst": xstage,
        "w1": what, "w2": w2b, "b1": b1c, "b2": b2c,
    }


_NC_CACHE = {}


def kernel(x, ind, W1, b1, W2, b2):
    from concourse import bass_utils

    cfg = Cfg()
    key = (cfg.n, cfg.np, cfg.mega)
    if key not in _NC_CACHE:
        _NC_CACHE[key] = build_program(cfg)
    nc = _NC_CACHE[key]

    x = np.asarray(x, np.float32)
    ind32 = np.asarray(ind).astype(np.int32)
    xb, what, w2b, b1c, b2c = host_prep(cfg, x, ind32, np.asarray(W1, np.float32),
                                        np.asarray(b1, np.float32),
                                        np.asarray(W2, np.float32),
                                        np.asarray(b2, np.float32))
    in_maps = []
    for c in range(NCORES):
        lo = c * NP
        hi = min(lo + NP, N)
        in_maps.append(core_inputs(cfg, xb, what, w2b, b1c, b2c, ind32, lo, hi))

    res = bass_utils.run_bass_kernel_spmd(nc, in_maps, core_ids=list(range(NCORES)))
    out = np.empty((N, H), np.float32)
    for c in range(NCORES):
        lo = c * NP
        hi = min(lo + NP, N)
        out[lo:hi] = res.results[c]["out2"].T[:hi - lo]
    return out



# revision 4
# speedup vs baseline: 1.3044x; 1.3044x over previous
"""EdgeConv (gnn_message_passing) Trainium2 Bass kernel, v2.

Computation (reference):
    neigh = x[ind]                                   # [n, k, d] gather
    feat  = [neigh - center, center]                 # [n, k, 2d]
    h     = relu(feat @ W1 + b1) @ W2 + b2           # [n, k, H]
    out   = max over k                               # [n, H]

Key ideas vs v1 (which used per-128-row indirect_dma_start gathers and a DMA
transpose; GPSIMD descriptor generation was 67% busy and the bottleneck):

  - One `dma_gather(transpose=True)` per 512-point block fetches all 8192
    neighbor rows straight into a feature-major slab [128, 8192] -- the gather
    and the transpose are fused into a single SWDGE instruction (994ns fixed
    cost amortized over 8192 rows instead of 64 separate indirect DMAs).
  - dma_gather indices are int16 and rows must be 256B, so the host stages a
    per-block COMPACTED table: unique neighbor x-rows of that block, padded to
    [x_j | zeros] 128 bf16 = 256B.  ~7.9k distinct rows per block << 32767.
  - Edges are laid out K-MAJOR within a block (col = k*NB + pt) so the center
    half of the slab (partitions 64..127) is written by one DVE copy with a
    stride-0 broadcast over k (out [64, K, NB] <- in [64, 1, NB]).
  - mm1 uses the repacked stationary [[W1[:d]], [W1[d:]-W1[:d]]], so
    slab = [neigh | center] needs no subtraction; b1 is the ACT relu bias.
  - k-max is a pairwise tensor_tensor(max) of neighboring k-stripe PSUM tiles
    (fp32, stage 1) followed by a bf16 SBUF max tree (stages 2-4), avoiding a
    full fp32 tensor_reduce pass over PSUM.
  - b2 is added on the host after the max (max(h)+b2 == max(h+b2)); output is
    returned bf16 feature-major and transposed/cast on the host.
"""

import os
import sys

for _p in ("/opt/trn_rl_repo",):
    if _p not in sys.path and os.path.isdir(_p):
        sys.path.insert(0, _p)

import numpy as np
import ml_dtypes

BF16 = ml_dtypes.bfloat16

# problem constants (hardcoded per harness contract)
N, D, K, H = 100000, 64, 16, 128
NCORES = 8
NP = 12500             # points per core
NB = 512               # points per block
EB = NB * K            # edges (gather indices) per block = 8192
TR = 8704              # compacted-table rows per block (>= max distinct + pad)


class Cfg:
    def __init__(self, n=N, np_=NP, nb=NB, tr=TR, gchunk=None,
                 single_packet=False):
        self.n = n
        self.np = np_                   # points handled by this core (unpadded)
        self.nb = nb                    # points per block
        self.eb = nb * K
        self.nblk = -(-np_ // nb)       # ceil
        self.npp = self.nblk * nb       # padded points per core
        self.tr = tr
        # gather call chunking: max indices per dma_gather call (multiple of
        # 128); None = whole block in one call
        self.gchunk = gchunk or self.eb
        self.single_packet = single_packet


def build_program(cfg: Cfg, debug=False):
    import concourse.bacc as bacc
    import concourse.bass as bass
    import concourse.tile as tile
    from concourse import mybir

    f32 = mybir.dt.float32
    bf16 = mybir.dt.bfloat16
    i16 = mybir.dt.int16
    NBK = cfg.nblk
    NBc = cfg.nb
    EBc = cfg.eb
    TRc = cfg.tr

    nc = bacc.Bacc("TRN2", target_bir_lowering=False, debug=debug)

    xtab = nc.dram_tensor("xtab", (NBK, TRc, 2 * D), bf16, kind="ExternalInput")
    idxt = nc.dram_tensor("idxt", (NBK, 128, EBc // 16), i16, kind="ExternalInput")
    xot = nc.dram_tensor("xot", (D, cfg.npp), bf16, kind="ExternalInput")
    w1 = nc.dram_tensor("w1", (2 * D, H), bf16, kind="ExternalInput")
    w2 = nc.dram_tensor("w2", (H, H), bf16, kind="ExternalInput")
    b1 = nc.dram_tensor("b1", (H, 1), f32, kind="ExternalInput")
    out2 = nc.dram_tensor("out2", (H, cfg.npp), bf16, kind="ExternalOutput")

    with tile.TileContext(nc) as tc:
        with (
            tc.tile_pool(name="const", bufs=1) as constp,
            tc.tile_pool(name="idx", bufs=3) as idxp,
            tc.tile_pool(name="xo", bufs=3) as xop,
            tc.tile_pool(name="slab", bufs=3) as slabp,
            tc.tile_pool(name="h1", bufs=3) as h1p,
            tc.tile_pool(name="s2", bufs=2) as s2p,
            tc.tile_pool(name="mx", bufs=2) as mxp,
            tc.tile_pool(name="mx4", bufs=2) as mx4p,
            tc.tile_pool(name="mx2", bufs=2) as mx2p,
            tc.tile_pool(name="outs", bufs=2) as outp,
            tc.tile_pool(name="ps1", bufs=2, space="PSUM") as ps1p,
            tc.tile_pool(name="ps2", bufs=2, space="PSUM") as ps2p,
        ):
            w1s = constp.tile([2 * D, H], bf16)
            nc.sync.dma_start(w1s[:], w1[:, :])
            w2s = constp.tile([H, H], bf16)
            nc.sync.dma_start(w2s[:], w2[:, :])
            b1s = constp.tile([H, 1], f32)
            nc.sync.dma_start(b1s[:], b1[:, :])

            for m in range(NBK):
                idx = idxp.tile([128, EBc // 16], i16)
                nc.sync.dma_start(idx[:], idxt[m])
                xo = xop.tile([D, NBc], bf16)
                nc.sync.dma_start(xo[:], xot[:, m * NBc:(m + 1) * NBc])

                slab = slabp.tile([128, EBc], bf16)
                # single_packet=True wedges the device above ~992 indices
                # (>64 descriptors per SDMA engine in one packet); use the
                # multi-packet path.
                for off in range(0, EBc, cfg.gchunk):
                    n = min(cfg.gchunk, EBc - off)
                    nc.gpsimd.dma_gather(
                        out_ap=slab[:, off:off + n].rearrange(
                            "p (a b) -> p a b", a=1),
                        in_ap=xtab[m],
                        idxs_ap=idx[:, off // 16:(off + n) // 16],
                        num_idxs=n,
                        num_idxs_reg=n,
                        elem_size=2 * D,
                        transpose=True,
                        single_packet=cfg.single_packet,
                    )
                # centers into the slab's upper half, broadcast over k
                nc.vector.tensor_copy(
                    out=slab[D:2 * D, :].rearrange("p (k b) -> p k b", k=K),
                    in_=xo[:].unsqueeze(1).broadcast_to((D, K, NBc)),
                )

                mx = mxp.tile([H, K // 2, NBc], bf16)
                for t in range(K // 2):
                    p1 = ps1p.tile([H, 2, NBc], f32)
                    nc.tensor.matmul(
                        p1[:, 0], lhsT=w1s[:],
                        rhs=slab[:, (2 * t) * NBc:(2 * t + 1) * NBc],
                        start=True, stop=True,
                    )
                    nc.tensor.matmul(
                        p1[:, 1], lhsT=w1s[:],
                        rhs=slab[:, (2 * t + 1) * NBc:(2 * t + 2) * NBc],
                        start=True, stop=True,
                    )
                    h1 = h1p.tile([H, 2, NBc], bf16)
                    nc.scalar.activation(
                        h1[:], p1[:], mybir.ActivationFunctionType.Relu,
                        bias=b1s[:], scale=1.0,
                    )
                    p2 = ps2p.tile([H, 2, NBc], f32)
                    nc.tensor.matmul(p2[:, 0], lhsT=w2s[:], rhs=h1[:, 0],
                                     start=True, stop=True)
                    nc.tensor.matmul(p2[:, 1], lhsT=w2s[:], rhs=h1[:, 1],
                                     start=True, stop=True)
                    # k-pair max; DVE TensorTensor cannot take two PSUM
                    # operands, so split pair-max between an ACT-evac route
                    # (ACT copy to bf16, DVE 4x max) and a DVE strided
                    # tensor_reduce route to balance ACT/DVE load.
                    if t < 3:
                        s2 = s2p.tile([H, 2, NBc], bf16)
                        nc.scalar.activation(
                            s2[:], p2[:], mybir.ActivationFunctionType.Copy,
                        )
                        nc.vector.tensor_tensor(
                            out=mx[:, t], in0=s2[:, 0], in1=s2[:, 1],
                            op=mybir.AluOpType.max,
                        )
                    else:
                        nc.vector.tensor_reduce(
                            out=mx[:, t],
                            in_=p2[:].transpose([0, 2, 1]),
                            axis=mybir.AxisListType.X,
                            op=mybir.AluOpType.max,
                        )

                mx4 = mx4p.tile([H, 4, NBc], bf16)
                nc.vector.tensor_tensor(out=mx4[:], in0=mx[:, 0:4],
                                        in1=mx[:, 4:8], op=mybir.AluOpType.max)
                mx2 = mx2p.tile([H, 2, NBc], bf16)
                nc.vector.tensor_tensor(out=mx2[:], in0=mx4[:, 0:2],
                                        in1=mx4[:, 2:4], op=mybir.AluOpType.max)
                outt = outp.tile([H, NBc], bf16)
                nc.vector.tensor_tensor(out=outt[:], in0=mx2[:, 0],
                                        in1=mx2[:, 1], op=mybir.AluOpType.max)
                nc.sync.dma_start(out2[:, m * NBc:(m + 1) * NBc], outt[:])

    nc.compile()
    return nc


def host_prep(x, W1, b1, W2, b2):
    """Shared (core-independent) input prep."""
    xb = np.ascontiguousarray(x.astype(BF16))
    what = np.vstack([W1[:D], W1[D:] - W1[:D]]).astype(BF16)
    w2b = W2.astype(BF16)
    b1c = np.ascontiguousarray(b1.astype(np.float32).reshape(H, 1))
    return xb, what, w2b, b1c


def core_inputs(cfg: Cfg, xb, what, w2b, b1c, ind32, lo, hi):
    """Build one core's input map for its point range [lo, hi)."""
    NBK, NBc, EBc, TRc = cfg.nblk, cfg.nb, cfg.eb, cfg.tr
    indc = np.zeros((cfg.npp, K), np.int64)
    indc[:hi - lo] = ind32[lo:hi]

    xtab = np.zeros((NBK, TRc, 2 * D), BF16)
    idxt = np.empty((NBK, 128, EBc // 16), np.int16)
    for b in range(NBK):
        blk = indc[b * NBc:(b + 1) * NBc]               # [NB, K]
        uniq, inv = np.unique(blk, return_inverse=True)
        r = len(uniq)
        assert r <= TRc, f"block {b}: {r} distinct rows > table {TRc}"
        xtab[b, :r, 0:D] = xb[uniq]
        # k-major edge order: col j = k*NB + pt
        cols = inv.reshape(NBc, K).T.reshape(EBc)       # [EB] int
        lanes = cols.reshape(EBc // 16, 16).T           # [16, EB/16]
        idxt[b] = np.tile(lanes.astype(np.int16), (8, 1))

    xo = np.zeros((D, cfg.npp), BF16)
    xo[:, :hi - lo] = xb[lo:hi].T
    return {
        "xtab": xtab, "idxt": idxt, "xot": np.ascontiguousarray(xo),
        "w1": what, "w2": w2b, "b1": b1c,
    }


_NC_CACHE = {}


def kernel(x, ind, W1, b1, W2, b2):
    from concourse import bass_utils

    cfg = Cfg()
    key = (cfg.n, cfg.np, cfg.nb, cfg.tr)
    if key not in _NC_CACHE:
        _NC_CACHE[key] = build_program(cfg)
    nc = _NC_CACHE[key]

    x = np.asarray(x, np.float32)
    ind32 = np.asarray(ind).astype(np.int64)
    xb, what, w2b, b1c = host_prep(x, np.asarray(W1, np.float32),
                                   np.asarray(b1, np.float32),
                                   np.asarray(W2, np.float32),
                                   np.asarray(b2, np.float32))
    in_maps = []
    for c in range(NCORES):
        lo = c * NP
        hi = min(lo + NP, N)
        in_maps.append(core_inputs(cfg, xb, what, w2b, b1c, ind32, lo, hi))

    res = bass_utils.run_bass_kernel_spmd(nc, in_maps, core_ids=list(range(NCORES)))
    b2f = np.asarray(b2, np.float32).reshape(1, H)
    out = np.empty((N, H), np.float32)
    for c in range(NCORES):
        lo = c * NP
        hi = min(lo + NP, N)
        out[lo:hi] = res.results[c]["out2"].T[:hi - lo].astype(np.float32) + b2f
    return out
